# revision 15
# baseline (speedup 1.0000x reference)
"""Self-contained Trainium2 Bass kernel for nn_CA_9363028705415 (sparse_attention).

Computes, per batch b:
    Q = relu(x[b] @ qW1 + qb1) @ qW2 + qb2          # [M, K]
    Kt = relu(x[b] @ kW1 + kb1) @ kW2 + kb2         # [M, K]
    S = Q @ Kt.T                                    # [M, M]
    out[b] = softmax(S / rowmax(S), axis=-1)        # max-DIVISION normalization

Shapes: B=16, M=2048, D=128, H=256, K=64.  Output [16, 2048, 2048] f32 (256 MB)
=> memory-bound on the output write (~32 MB/core across 8 cores).

Sharding: data-parallel over batch across 8 NeuronCores; 2 batches/core; tiny
MLP weights replicated.  Single NEFF run SPMD via run_bass_kernel_spmd.

Pipeline per 128-row tile (per-engine, overlapped):
  PE:  S = Q K^T into a 4-bank PSUM tile (4 x N=512 bf16 matmuls)
  DVE: fused PSUM->SBUF fp16 copy + row-max (tensor_scalar accum_out=max),
       frees the PSUM slot early; reciprocal of max
  ACT: exp(S * (1/max)) from the SBUF copy, fused row-sum accumulate
  DVE/ACT: multiply by 1/rowsum into the staging tile (split by NORM_PATTERN)
  HWDGE DMA: 2 MB output chunks
The next batch's MLP/transpose work is emitted in small chunks interleaved
into the current batch's S loop so the per-engine in-order queues never see a
long MLP bubble.
"""

import numpy as np
import ml_dtypes

import concourse.bass as bass
import concourse.mybir as mybir
from concourse import bacc
import concourse.tile as tile
from concourse.bass import ts
from concourse.bass_utils import run_bass_kernel_spmd

F32 = mybir.dt.float32
BF16 = mybir.dt.bfloat16
FP16 = mybir.dt.float16
AF = mybir.ActivationFunctionType
ALU = mybir.AluOpType

N_CORES = 8
B, M, D, H, KF = 16, 2048, 128, 256, 64
BPC = B // N_CORES     # batches per core
MT = M // 128          # 16 row-tiles per batch
FC = M // 512          # 4 matmul free-chunks of 512
PAIR = 2               # row-tiles per output DMA (2 MB chunks)

# normalize engine per row-tile: DVE tensor_scalar is 2x fp32 SBUF
# (1.28us/tile), ACT copy-with-scale is 1x (2.06us/tile).  gpsimd is NOT used:
# its software tensor_scalar is ~29us/tile AND locks the DVE-shared SBUF port.
NORM_PATTERN = (
    "dve", "act", "dve", "act", "dve", "act", "dve", "act",
    "dve", "act", "dve", "act", "dve", "act", "dve", "dve",
)
RELU_ENGINES = ("act", "dve", "act", "dve")  # hT evac per (head, pc)
QT_EVAC = "dve"
KT_EVAC = "act"


def _evac_bias(nc, engine, out, in_, bias, relu):
    """out = [relu](in_ + bias), bias is [P,1] per-partition AP."""
    if engine == "act":
        nc.scalar.activation(
            out, in_, AF.Relu if relu else AF.Identity, bias=bias, scale=1.0
        )
    else:
        if relu:
            nc.vector.tensor_scalar(out, in_, bias, 0.0, op0=ALU.add, op1=ALU.max)
        else:
            nc.vector.tensor_scalar(out, in_, bias, None, op0=ALU.add)


def _norm(nc, engine, out, t, isum):
    if engine == "act":
        nc.scalar.mul(out, t, isum)
    else:
        nc.vector.tensor_scalar_mul(out, t, isum)


def build_nc():
    nc = bacc.Bacc()

    x = nc.dram_tensor("x", [BPC, M, D], F32, kind="ExternalInput")
    w1d, b1d, w2d, b2d = {}, {}, {}, {}
    for h in ("q", "k"):
        w1d[h] = nc.dram_tensor(f"{h}W1", [D, H], F32, kind="ExternalInput")
        b1d[h] = nc.dram_tensor(f"{h}b1", [H], F32, kind="ExternalInput")
        w2d[h] = nc.dram_tensor(f"{h}W2", [H, KF], F32, kind="ExternalInput")
        b2d[h] = nc.dram_tensor(f"{h}b2", [KF], F32, kind="ExternalInput")
    out = nc.dram_tensor("out", [BPC, M, M], F32, kind="ExternalOutput")

    ident_np = np.eye(128, dtype=ml_dtypes.bfloat16)
    ident_dram = nc.inline_tensor(ident_np, name="ident_data")

    # [b, p, n, d]: token (n*128+p), feature d
    x_r = x[:].rearrange("b (n p) d -> b p n d", p=128)
    # [b, p, n, m]: out[b, n*128+p, m]
    out_r = out[:].rearrange("b (n p) m -> b p n m", p=128)

    with tile.TileContext(nc) as tc:
        with (
            tc.tile_pool(name="consts", bufs=1) as consts,
            tc.tile_pool(name="xin", bufs=2) as xin_pool,
            tc.tile_pool(name="xt", bufs=2) as xt_pool,
            tc.tile_pool(name="ht", bufs=2) as ht_pool,
            tc.tile_pool(name="qkt", bufs=2) as qkt_pool,
            tc.tile_pool(name="texp", bufs=3) as t_pool,
            tc.tile_pool(name="osb", bufs=3) as out_pool,
            tc.tile_pool(name="small", bufs=6) as small_pool,
            tc.tile_pool(name="psum", bufs=2, space="PSUM") as psum_pool,
        ):
            norm_i = 0

            # ---- x loads for batch 0 first: nothing can start without x ----
            xf = {}
            for b in range(BPC):
                xf[b] = xin_pool.tile([128, MT, 128], F32, tag=f"xf{b}", name="xf")
            for g in range(2):
                nc.sync.dma_start(
                    out=xf[0][:, g * 8 : (g + 1) * 8, :],
                    in_=x_r[0][:, g * 8 : (g + 1) * 8, :],
                )

            # ---- constants (identity from inline data; weights cast) ----
            ident = consts.tile([128, 128], BF16, tag="ident")
            nc.sync.dma_start(out=ident, in_=ident_dram[:])
            w1, w2, b1, b2 = {}, {}, {}, {}
            for h in ("q", "k"):
                w1[h] = consts.tile([D, H], BF16, tag=f"w1{h}", name=f"w1{h}")
                nc.gpsimd.dma_start(out=w1[h], in_=w1d[h][:])  # cast f32->bf16
                w2[h] = consts.tile([128, 2, KF], BF16, tag=f"w2{h}", name=f"w2{h}")
                nc.gpsimd.dma_start(
                    out=w2[h], in_=w2d[h][:].rearrange("(c p) k -> p c k", p=128)
                )
                b1[h] = consts.tile([128, 2], F32, tag=f"b1{h}", name=f"b1{h}")
                nc.sync.dma_start(
                    out=b1[h], in_=b1d[h][:].rearrange("(c p) -> p c", p=128)
                )
                b2[h] = consts.tile([KF, 1], F32, tag=f"b2{h}", name=f"b2{h}")
                nc.sync.dma_start(
                    out=b2[h], in_=b2d[h][:].rearrange("(k o) -> k o", o=1)
                )
            for g in range(2):
                nc.sync.dma_start(
                    out=xf[1][:, g * 8 : (g + 1) * 8, :],
                    in_=x_r[1][:, g * 8 : (g + 1) * 8, :],
                )

            def phase_a_chunks(b, fast=False):
                """Phase-A emission chunks for batch b, fine-grained so the
                serial ramp is short and chunks interleave into the previous
                batch's S loop.  Evac engines alternate DVE/ACT."""
                ctx = {}

                def c_cast_tp(g, eng):
                    def go():
                        if "xsb" not in ctx:
                            ctx["xsb"] = xin_pool.tile(
                                [128, MT, 128], BF16, tag=f"x{b}", name="xsb"
                            )
                            ctx["xT"] = xt_pool.tile(
                                [128, M], BF16, tag="xt", name="xT"
                            )
                        nc.vector.tensor_copy(
                            ctx["xsb"][:, g * 8 : (g + 1) * 8, :],
                            xf[b][:, g * 8 : (g + 1) * 8, :],
                        )
                        tp = psum_pool.tile([128, 1024], BF16, tag="ps", name="tp")
                        for it in range(8):
                            nc.tensor.transpose(
                                tp[:, ts(it, 128)], ctx["xsb"][:, g * 8 + it, :], ident
                            )
                        if eng == "dve":
                            nc.vector.tensor_copy(ctx["xT"][:, ts(g, 1024)], tp)
                        else:
                            nc.scalar.copy(ctx["xT"][:, ts(g, 1024)], tp)
                    return go

                def c_mlp1(h, pc, half, eng):
                    def go():
                        if ("ht", h) not in ctx:
                            ctx[("ht", h)] = ht_pool.tile(
                                [128, 2, M], BF16, tag=f"ht{h}", name=f"ht{h}"
                            )
                        ps1 = psum_pool.tile([128, 1024], F32, tag="ps", name="ps1")
                        for fc in range(2):
                            nc.tensor.matmul(
                                ps1[:, ts(fc, 512)],
                                lhsT=w1[h][:, ts(pc, 128)],
                                rhs=ctx["xT"][:, ts(half * 2 + fc, 512)],
                                start=True,
                                stop=True,
                            )
                        if fast:
                            # split the evac across both engines (ramp latency)
                            for e, fc in (("act", 0), ("dve", 1)):
                                _evac_bias(
                                    nc,
                                    e,
                                    ctx[("ht", h)][:, pc, ts(half * 2 + fc, 512)],
                                    ps1[:, ts(fc, 512)],
                                    b1[h][:, pc : pc + 1],
                                    relu=True,
                                )
                        else:
                            _evac_bias(
                                nc,
                                eng,
                                ctx[("ht", h)][:, pc, ts(half, 1024)],
                                ps1,
                                b1[h][:, pc : pc + 1],
                                relu=True,
                            )
                    return go

                def c_mlp2(h):
                    def go():
                        ps2 = psum_pool.tile([KF, M], F32, tag="ps", name="ps2")
                        for fc in range(FC):
                            for kc in range(2):
                                nc.tensor.matmul(
                                    ps2[:, ts(fc, 512)],
                                    lhsT=w2[h][:, kc, :],
                                    rhs=ctx[("ht", h)][:, kc, ts(fc, 512)],
                                    start=(kc == 0),
                                    stop=(kc == 1),
                                )
                        q = qkt_pool.tile([KF, M], BF16, tag=f"qkt{h}", name=f"qkt{h}")
                        ctx[("qkt", h)] = q
                        # chunked evac so the first S matmuls can start on
                        # column chunk 0 before the rest are evacuated
                        for fc in range(FC):
                            _evac_bias(
                                nc,
                                ("act", "dve")[fc % 2] if fast
                                else (QT_EVAC if h == "q" else KT_EVAC),
                                q[:, ts(fc, 512)],
                                ps2[:, ts(fc, 512)],
                                b2[h],
                                relu=False,
                            )
                    return go

                chunks = [c_cast_tp(g, ("dve", "act")[g % 2]) for g in range(2)]
                for i, (h, pc, half) in enumerate(
                    [
                        ("q", 0, 0), ("k", 0, 0), ("q", 0, 1), ("k", 0, 1),
                        ("q", 1, 0), ("k", 1, 0), ("q", 1, 1), ("k", 1, 1),
                    ]
                ):
                    chunks.append(c_mlp1(h, pc, half, ("act", "dve")[i % 2]))
                chunks.append(c_mlp2("q"))
                chunks.append(c_mlp2("k"))
                return ctx, chunks

            def s_loop(b, qkt, next_chunks):
                """Emit the S+softmax loop for batch b, interleaving
                next_chunks (next batch's phase A) into the early iterations."""
                nonlocal norm_i
                osb_tiles = {}
                pending = None

                def finish(j, t_j, isum_ap):
                    nonlocal norm_i
                    _norm(
                        nc,
                        NORM_PATTERN[norm_i % len(NORM_PATTERN)],
                        osb_tiles[j // PAIR][:, ts(j % PAIR, M)],
                        t_j,
                        isum_ap,
                    )
                    norm_i += 1
                    if j % PAIR == PAIR - 1:
                        osb = osb_tiles.pop(j // PAIR)
                        if j == MT - 1:
                            for jj in range(PAIR):
                                nc.sync.dma_start(
                                    out=out_r[b][:, j - PAIR + 1 + jj : j - PAIR + 2 + jj, :],
                                    in_=osb[:, ts(jj, M)],
                                )
                        else:
                            nc.sync.dma_start(
                                out=out_r[b][:, j - PAIR + 1 : j + 1, :],
                                in_=osb,
                            )

                # pairs[rt] holds [row-max(rt) | exp-row-sum(rt-1)]; one
                # reciprocal per tile covers both 1/max(rt) and 1/sum(rt-1).
                pairs = {0: small_pool.tile([128, 2], F32, tag="pr", name="pair")}
                nc.vector.memset(pairs[0], 1.0)
                for rt in range(MT):
                    ps_s = psum_pool.tile([128, M], F32, tag="ps", name="ps_s")
                    for fc in range(FC):
                        nc.tensor.matmul(
                            ps_s[:, ts(fc, 512)],
                            lhsT=qkt["q"][:, ts(rt, 128)],
                            rhs=qkt["k"][:, ts(fc, 512)],
                            start=True,
                            stop=True,
                        )
                    # Evacuate S from PSUM to fp16 SBUF with fused row-max
                    # (tensor_scalar accum_out reduces with op1); frees the
                    # PSUM slot so exp reads the SBUF copy instead.
                    sc_t = t_pool.tile([128, M], FP16, tag="sc", name="sc")
                    nc.vector.tensor_scalar(
                        sc_t,
                        ps_s,
                        0.0,
                        None,
                        op0=ALU.add,
                        op1=ALU.max,
                        accum_out=pairs[rt][:, 0:1],
                    )

                    ipair = small_pool.tile([128, 2], F32, tag="ip", name="ipair")
                    nc.vector.reciprocal(ipair, pairs[rt])
                    pairs[rt + 1] = small_pool.tile([128, 2], F32, tag="pr", name="pair")

                    t_t = t_pool.tile([128, M], FP16, tag="t")
                    nc.scalar.activation(
                        t_t,
                        sc_t,
                        AF.Exp,
                        bias=0.0,
                        scale=ipair[:, 0:1],
                        accum_out=pairs[rt + 1][:, 1:2],
                    )

                    if rt % PAIR == 0:
                        osb_tiles[rt // PAIR] = out_pool.tile(
                            [128, PAIR * M], F32, tag="o", name="osb"
                        )
                    if pending is not None:
                        finish(pending[0], pending[1], ipair[:, 1:2])
                    pending = (rt, t_t)

                    # interleave the next batch's MLP work
                    if next_chunks:
                        next_chunks.pop(0)()
                last_is = small_pool.tile([128, 1], F32, tag="li", name="last_is")
                nc.vector.reciprocal(last_is, pairs[MT][:, 1:2])
                finish(pending[0], pending[1], last_is)
                while next_chunks:
                    next_chunks.pop(0)()

            ctx0, chunks0 = phase_a_chunks(0, fast=True)
            for c in chunks0:
                c()
            qkt0 = {"q": ctx0[("qkt", "q")], "k": ctx0[("qkt", "k")]}

            ctx1, chunks1 = phase_a_chunks(1)
            s_loop(0, qkt0, chunks1)
            qkt1 = {"q": ctx1[("qkt", "q")], "k": ctx1[("qkt", "k")]}
            s_loop(1, qkt1, [])
    nc.finalize()
    return nc


_NC_CACHE = None


def _get_nc():
    global _NC_CACHE
    if _NC_CACHE is None:
        _NC_CACHE = build_nc()
    return _NC_CACHE


def run(inputs, trace=False, trace_cores=None):
    """Run on 8 cores; returns (full_output [B,M,M] f32, BassKernelResults)."""
    nc = _get_nc()
    in_maps = []
    x = np.ascontiguousarray(inputs["x"], dtype=np.float32)
    for c in range(N_CORES):
        im = {"x": np.ascontiguousarray(x[c * BPC : (c + 1) * BPC])}
        for k in ("qW1", "qb1", "qW2", "qb2", "kW1", "kb1", "kW2", "kb2"):
            im[k] = np.ascontiguousarray(inputs[k], dtype=np.float32)
        in_maps.append(im)
    res = run_bass_kernel_spmd(
        nc,
        in_maps,
        core_ids=list(range(N_CORES)),
        trace=trace,
        trace_cores=trace_cores,
    )
    outs = [r["out"] for r in res.results]
    full = np.concatenate(outs, axis=0)
    assert full.shape == (B, M, M) and full.dtype == np.float32
    return full, res


def kernel(**inputs) -> np.ndarray:
    out, _ = run(inputs, trace=False)
    return out
